# revision 16
# baseline (speedup 1.0000x reference)
"""Trainium2 Bass kernel for nn_DecoderRNN (LSTM decoder + big vocab projection).

Strategy (8 NeuronCores, SPMD):
  - The LSTM recurrence (B=32, T=64, H=512) is replicated on every core:
    its per-step cost is dominated by streaming W_hh through the PE, which is
    batch-size independent, so sharding batch would not help. Replication
    keeps every core self-sufficient (no collectives).
  - The output projection (fc) is tensor-parallel over the vocab dim:
    core c computes logits[:, :, 1250*c : 1250*(c+1)] and writes its own
    [32, 64, 1250] output slab; the host concatenates slabs.
  - Embedding lookup runs on-device via indirect (gather) DMA.
  - The input projection Xp = xs @ W_ih.T + b is computed in bulk up front
    (it has no recurrent dependency) and bounced through a DRAM scratch
    buffer, prefetched per-step during the recurrence.

Layouts:
  - Gates are computed in batch-layout [B=32, 4H] PSUM via
    out = h.T_tile.T @ W_hh.T  (stationary = h.T tiles, moving = W_hh.T),
    using float32r (full fp32 storage, 1 cycle/row at N>=256).
  - Gate column order is permuted host-side to [i | f | o | g] so one
    sigmoid activation covers cols [0:1536) and one tanh covers [1536:2048).
  - h is transposed each step via PE transpose-mode into hsT (h.T history),
    which doubles as the stationary operand for both the recurrence and fc.

kernel(**inputs) takes FULL unsharded inputs, returns FULL [32, 64, 10000].
"""

import sys

sys.path.insert(0, "/opt/trn_rl_repo")

import numpy as np

N_CORES = 8
B, T = 32, 64
E, H, V = 512, 512, 10000
G4 = 4 * H            # 2048
TB = T * B            # 2048
VSL = V // N_CORES    # 1250 vocab rows per core
VPAD = 1280           # padded so fc N-chunks are 512/512/256 (all >=256)

_PROGRAM = None


def _build_program():
    import concourse.bass as bass
    import concourse.tile as tile
    from concourse import bacc, mybir
    from concourse.masks import make_identity
    from contextlib import ExitStack

    f32 = mybir.dt.float32
    f32r = mybir.dt.float32r
    i32 = mybir.dt.int32
    AF = mybir.ActivationFunctionType

    nc = bacc.Bacc(
        "TRN2",
        target_bir_lowering=False,
        debug=False,
        num_devices=N_CORES,
    )

    features = nc.dram_tensor("features", [B, E], f32, kind="ExternalInput").ap()
    idx = nc.dram_tensor("idx", [TB], i32, kind="ExternalInput").ap()
    embed = nc.dram_tensor("embed", [V, E], f32, kind="ExternalInput").ap()
    wihT = nc.dram_tensor("wihT", [E, G4], f32, kind="ExternalInput").ap()
    whhT = nc.dram_tensor("whhT", [H, G4], f32, kind="ExternalInput").ap()
    bih = nc.dram_tensor("bih", [G4], f32, kind="ExternalInput").ap()
    bhh = nc.dram_tensor("bhh", [G4], f32, kind="ExternalInput").ap()
    fcwT = nc.dram_tensor("fcwT", [H, VPAD], f32, kind="ExternalInput").ap()
    fcb = nc.dram_tensor("fcb", [VPAD], f32, kind="ExternalInput").ap()
    onesv = nc.dram_tensor("onesv", [128], f32, kind="ExternalInput").ap()
    out = nc.dram_tensor("out", [B, T, VSL], f32, kind="ExternalOutput").ap()
    # Output viewed as [t, b, v] so a 128-row t-major tb tile maps to 4
    # consecutive t planes.
    out_r = out.rearrange("b t v -> t b v")

    with tile.TileContext(nc) as tc, ExitStack() as ctx:
        # ---------------- persistent state ----------------
        state = ctx.enter_context(tc.tile_pool(name="state", bufs=1))
        # h.T history: block t holds h(t).T (written at the end of step t).
        # Layout [p, k, 32*t + b] = h(t)[b, 128*k + p]
        hsT = state.tile([128, 4, 32 * T], f32r, tag="hsT")
        whhT_sb = state.tile([128, 4, G4], f32r, tag="whhT")
        c_sb = state.tile([B, H], f32, tag="c")
        ident = state.tile([128, 128], f32, tag="ident")
        ones = state.tile([1, 128], f32r, tag="ones")

        make_identity(nc, ident[:])
        nc.vector.memset(c_sb[:], 0.0)

        nc.sync.dma_start(whhT_sb[:], whhT.rearrange("(k p) g -> p k g", p=128).bitcast(f32r))
        nc.sync.dma_start(ones[:], onesv[None, :].bitcast(f32r))

        # DRAM scratch for the bulk input projection (t-major rows)
        dram = ctx.enter_context(tc.tile_pool(name="dram", bufs=1, space="DRAM"))
        xp_dram = dram.tile([TB, G4], f32)

        # ---------------- prologue: gather + xs.T + bulk Xp ----------------
        with ExitStack() as pro:
            xsT_pool = pro.enter_context(tc.tile_pool(name="xsT", bufs=1))
            small_pool = pro.enter_context(tc.tile_pool(name="small", bufs=1))
            tp_psum = pro.enter_context(tc.tile_pool(name="tp_ps", bufs=2, space="PSUM"))
            xp_psum = pro.enter_context(tc.tile_pool(name="xp_ps", bufs=2, space="PSUM"))
            xp_stage = pro.enter_context(tc.tile_pool(name="xp_st", bufs=3))

            xsT = xsT_pool.tile([128, 4, TB], f32r)        # [p, k, tb]: xs[tb, 128k+p]
            idx_sb = small_pool.tile([128, 16], i32, tag="idx")
            bias_sb = small_pool.tile([1, G4], f32r, tag="bias")
            bias1_sb = small_pool.tile([1, G4], f32, tag="bias1")
            bias2_sb = small_pool.tile([1, G4], f32, tag="bias2")

            nc.sync.dma_start(idx_sb[:], idx.rearrange("(m p) -> p m", p=128))
            nc.sync.dma_start(bias1_sb[:], bih[None, :])
            nc.sync.dma_start(bias2_sb[:], bhh[None, :])
            nc.vector.tensor_add(bias_sb[:], bias1_sb[:], bias2_sb[:])

            with ExitStack() as pro2:
                xs_pool = pro2.enter_context(tc.tile_pool(name="xs", bufs=1))
                xs = xs_pool.tile([128, 16, E], f32)      # [p, m, e]: row 128m+p

                # Embedding gather: xs row tb = embed[idx[tb]] (idx[0:32]
                # dummy 0), then overwrite rows 0..31 (t=0) with features.
                for m in range(16):
                    nc.gpsimd.indirect_dma_start(
                        out=xs[:, m, :],
                        out_offset=None,
                        in_=embed[:, :],
                        in_offset=bass.IndirectOffsetOnAxis(
                            ap=idx_sb[:, m : m + 1], axis=0
                        ),
                    )
                nc.sync.dma_start(xs[0:32, 0, :], features[:, :])

                # xs -> xs.T via PE transpose (64 [128,128] tiles)
                for m in range(16):
                    for e in range(4):
                        pt = tp_psum.tile([128, 128], f32, tag="pt")
                        nc.tensor.transpose(
                            pt[:], xs[:, m, 128 * e : 128 * (e + 1)], ident[:]
                        )
                        nc.vector.tensor_copy(xsT[:, e, 128 * m : 128 * (m + 1)], pt[:])

            # xs freed; W_ih.T loads into the reclaimed space
            wih_pool = pro.enter_context(tc.tile_pool(name="wih", bufs=1))
            wihT_sb = wih_pool.tile([128, 4, G4], f32r)
            nc.sync.dma_start(wihT_sb[:], wihT.rearrange("(k p) g -> p k g", p=128).bitcast(f32r))

            # Bulk Xp: [tb, gate] = xs @ W_ih.T + (b_ih + b_hh), to DRAM.
            # m-order ascending so early t rows land first for the recurrence.
            for m in range(16):
                for cch in range(4):
                    sl = slice(512 * cch, 512 * (cch + 1))
                    ps = xp_psum.tile([128, 512], f32, tag="xp")
                    for k in range(4):
                        nc.tensor.matmul(
                            ps[:],
                            lhsT=xsT[:, k, 128 * m : 128 * (m + 1)],
                            rhs=wihT_sb[:, k, sl],
                            start=(k == 0),
                            stop=False,
                        )
                    # + bias as a rank-1 K=1 matmul: ones.T @ bias_row
                    nc.tensor.matmul(
                        ps[:],
                        lhsT=ones[0:1, :],
                        rhs=bias_sb[0:1, sl],
                        start=False,
                        stop=True,
                    )
                    st = xp_stage.tile([128, 512], f32, tag="st")
                    nc.vector.tensor_copy(st[:], ps[:])
                    nc.sync.dma_start(
                        xp_dram[128 * m : 128 * (m + 1), sl],
                        st[:],
                    )

        # ---------------- main recurrence + interleaved fc ----------------
        fcw_pool = ctx.enter_context(tc.tile_pool(name="fcw", bufs=1))
        fcwT_sb = fcw_pool.tile([128, 4, VPAD], f32r, tag="fcwT")
        fcb_sb = fcw_pool.tile([1, VPAD], f32r, tag="fcb")
        nc.sync.dma_start(fcwT_sb[:], fcwT.rearrange("(k p) v -> p k v", p=128).bitcast(f32r))
        nc.sync.dma_start(fcb_sb[:], fcb[None, :].bitcast(f32r))

        xp_pool = ctx.enter_context(tc.tile_pool(name="xp_t", bufs=3))
        work = ctx.enter_context(tc.tile_pool(name="work", bufs=2))
        g_psum = ctx.enter_context(tc.tile_pool(name="g_ps", bufs=1, space="PSUM"))
        h_psum = ctx.enter_context(tc.tile_pool(name="h_ps", bufs=2, space="PSUM"))
        fc_psum = ctx.enter_context(tc.tile_pool(name="fc_ps", bufs=2, space="PSUM"))
        lg_pool = ctx.enter_context(tc.tile_pool(name="lg", bufs=2))

        # physical [128, VPAD] fc bias tile (broadcast via rank-1 matmul)
        fcb128 = fcw_pool.tile([128, VPAD], f32, tag="fcb128")
        for c0, csz in ((0, 512), (512, 512), (1024, 256)):
            bps = fc_psum.tile([128, 512], f32, tag="fc")
            nc.tensor.matmul(
                bps[:, 0:csz],
                lhsT=ones[0:1, :],
                rhs=fcb_sb[0:1, c0 : c0 + csz],
                start=True,
                stop=True,
            )
            nc.vector.tensor_copy(fcb128[:, c0 : c0 + csz], bps[:, 0:csz])

        # gate chunk order in SBUF columns (host permutes): 0=i 1=f 2=o 3=g
        # matmul issue order: g first so tanh(g) overlaps remaining matmuls.
        CHUNK_ORDER = (3, 0, 1, 2)

        for t in range(T):
            xp_t = xp_pool.tile([B, G4], f32, tag="xp_t")
            nc.sync.dma_start(xp_t[:], xp_dram[32 * t : 32 * (t + 1), :])

            gps = g_psum.tile([B, G4], f32, tag="g")
            gt = work.tile([B, G4], f32, tag="gates")
            nl = work.tile([B, G4], f32, tag="nl")
            for cch in CHUNK_ORDER:
                sl = slice(512 * cch, 512 * (cch + 1))
                if t == 0:
                    # h(-1) = 0: gates are just the input projection
                    nc.vector.tensor_copy(gt[:, sl], xp_t[:, sl])
                else:
                    for k in range(4):
                        nc.tensor.matmul(
                            gps[:, sl],
                            lhsT=hsT[:, k, 32 * (t - 1) : 32 * t],
                            rhs=whhT_sb[:, k, sl],
                            start=(k == 0),
                            stop=(k == 3),
                        )
                    nc.vector.tensor_add(gt[:, sl], gps[:, sl], xp_t[:, sl])
                if cch == 3:
                    nc.scalar.activation(nl[:, sl], gt[:, sl], AF.Tanh)
                else:
                    nc.scalar.activation(nl[:, sl], gt[:, sl], AF.Sigmoid)

            # c = sigmoid(f)*c + sigmoid(i)*tanh(g);  h = sigmoid(o)*tanh(c)
            ig = work.tile([B, H], f32, tag="ig")
            nc.gpsimd.tensor_mul(ig[:], nl[:, 0:512], nl[:, 1536:2048])
            fmul = work.tile([B, H], f32, tag="fmul")
            nc.gpsimd.tensor_mul(fmul[:], nl[:, 512:1024], c_sb[:])
            nc.vector.tensor_add(c_sb[:], fmul[:], ig[:])
            tanhc = work.tile([B, H], f32, tag="tanhc")
            nc.scalar.activation(tanhc[:], c_sb[:], AF.Tanh)
            h_t = work.tile([B, H], f32, tag="h")
            nc.vector.tensor_mul(h_t[:], nl[:, 1024:1536], tanhc[:])

            # h -> h.T into hsT block t+1 (4 PE transposes + one strided copy)
            hp = h_psum.tile([128, 128], f32, tag="hp")
            for k in range(4):
                nc.tensor.transpose(
                    hp[:, 32 * k : 32 * (k + 1)],
                    h_t[0:32, 128 * k : 128 * (k + 1)],
                    ident[0:32, 0:32],
                )
            nc.vector.tensor_copy(
                hsT[:, :, 32 * t : 32 * (t + 1)],
                hp[:].rearrange("p (k b) -> p k b", k=4),
            )

            # fc for a finished 128-row tb tile every 4 steps
            if (t + 1) % 4 == 0:
                m = (t + 1) // 4 - 1
                lg = lg_pool.tile([128, VPAD], f32, tag="lg")
                for c0, csz in ((0, 512), (512, 512), (1024, 256)):
                    fps = fc_psum.tile([128, 512], f32, tag="fc")
                    for k in range(4):
                        nc.tensor.matmul(
                            fps[:, 0:csz],
                            lhsT=hsT[:, k, 128 * m : 128 * (m + 1)],
                            rhs=fcwT_sb[:, k, c0 : c0 + csz],
                            start=(k == 0),
                            stop=(k == 3),
                        )
                    nc.vector.tensor_add(
                        lg[:, c0 : c0 + csz],
                        fps[:, 0:csz],
                        fcb128[:, c0 : c0 + csz],
                    )
                # DRAM side is [4 t, 32 b, 1250 v]; SBUF side [128, 1250] pairs
                # with it element-stream-wise (partition p = 32*t_local + b).
                nc.sync.dma_start(out_r[4 * m : 4 * (m + 1), :, :], lg[:, 0:VSL])

    nc.compile()
    return nc


def _get_program():
    global _PROGRAM
    if _PROGRAM is None:
        _PROGRAM = _build_program()
    return _PROGRAM


# PyTorch LSTM gate order is [i, f, g, o]; we reorder rows to [i, f, o, g] so
# one device-side sigmoid covers a contiguous [0:1536) column range.
def _gate_perm():
    return np.concatenate(
        [np.arange(0, H), np.arange(H, 2 * H), np.arange(3 * H, 4 * H), np.arange(2 * H, 3 * H)]
    )


def _make_in_maps(features, captions, embed_table, W_ih, W_hh, b_ih, b_hh, fc_W, fc_b):
    perm = _gate_perm()
    features = np.ascontiguousarray(np.asarray(features, dtype=np.float32))
    cap = np.asarray(captions).astype(np.int32)                      # [B, T]
    embed = np.ascontiguousarray(np.asarray(embed_table, dtype=np.float32))
    wihT = np.ascontiguousarray(np.asarray(W_ih, dtype=np.float32)[perm].T)  # [E, 4H]
    whhT = np.ascontiguousarray(np.asarray(W_hh, dtype=np.float32)[perm].T)  # [H, 4H]
    bih = np.ascontiguousarray(np.asarray(b_ih, dtype=np.float32)[perm])
    bhh = np.ascontiguousarray(np.asarray(b_hh, dtype=np.float32)[perm])
    fc_W = np.asarray(fc_W, dtype=np.float32)
    fc_b = np.asarray(fc_b, dtype=np.float32)

    # gather indices, t-major: xs row t*32+b = embed[captions[b, t-1]] for t>=1
    idx = np.zeros(TB, dtype=np.int32)
    idx[B:] = cap[:, : T - 1].T.reshape(-1)

    in_maps = []
    for c in range(N_CORES):
        sl = slice(VSL * c, VSL * (c + 1))
        fcwT = np.zeros((H, VPAD), dtype=np.float32)
        fcwT[:, :VSL] = fc_W[sl].T
        fcbp = np.zeros(VPAD, dtype=np.float32)
        fcbp[:VSL] = fc_b[sl]
        in_maps.append(
            dict(
                features=features,
                idx=idx,
                embed=embed,
                wihT=wihT,
                whhT=whhT,
                bih=bih,
                bhh=bhh,
                fcwT=np.ascontiguousarray(fcwT),
                fcb=fcbp,
                onesv=np.ones(128, dtype=np.float32),
            )
        )
    return in_maps


def _install_ntff_hook():
    """Wire up NTFF profiling: bass_utils wants antenv.axon_hooks, which this
    container lacks; build it from trn_agent_boot's ctypes hook."""
    import sys as _sys
    import types

    if "antenv.axon_hooks" in _sys.modules:
        return
    if "/root/.axon_site" not in _sys.path:
        _sys.path.insert(0, "/root/.axon_site")
    from trn_agent_boot.trn_boot import _ntff_profile_via_ctypes

    hook = _ntff_profile_via_ctypes("/opt/axon/libaxon_pjrt.so")
    mod = types.ModuleType("antenv.axon_hooks")
    mod._hook = hook
    mod.set_axon_ntff_profile_hook = lambda h: setattr(mod, "_hook", h)
    mod.get_axon_ntff_profile_hook = lambda: mod._hook
    _sys.modules["antenv.axon_hooks"] = mod

    # avoid S3 uploads from the trace path in this zero-egress container
    import concourse.bass_utils as bu

    bu.upload_artifacts = lambda tmpdir: f"local:{tmpdir}"


def run(inputs, trace=False, trace_cores=None):
    """Run on hardware; returns (full_output [B,T,V] f32, BassKernelResults)."""
    from concourse.bass_utils import run_bass_kernel_spmd

    if trace:
        _install_ntff_hook()

    nc = _get_program()
    in_maps = _make_in_maps(
        inputs["features"],
        inputs["captions"],
        inputs["embed_table"],
        inputs["W_ih"],
        inputs["W_hh"],
        inputs["b_ih"],
        inputs["b_hh"],
        inputs["fc_W"],
        inputs["fc_b"],
    )
    kwargs = {}
    if trace:
        kwargs.update(trace=True, trace_cores=trace_cores or [0])
    res = run_bass_kernel_spmd(nc, in_maps, core_ids=list(range(N_CORES)), **kwargs)
    full = np.concatenate([r["out"] for r in res.results], axis=2)
    return full, res


def kernel(**inputs) -> np.ndarray:
    out, _ = run(inputs, trace=False)
    return out
